# revision 15
# baseline (speedup 1.0000x reference)
"""Trainium2 Bass kernel for bag-level attention (ragged_sequence).

Math (per bag b over its sentences i):
    att_i  = <x_i, rel[q_i]>
    w      = softmax(att) within bag
    logits = (sum_i w_i x_i) @ rel.T + bias

Key identity used: logits[b] = sum_i w_i (x_i @ rel.T) + bias = sum_i w_i S[i,:] + bias
with S = x @ rel.T, so x is read from HBM exactly once.

Device layout (per core, rows = N/8 sentences, bags of 16):
    S.T [53, rows] = relT.T @ xT on TensorE (relT stationary, xT streamed).
    att [1, rows]  = ones53.T @ (S.T * onehotT)      (one-hot built on host)
    e    = exp(att)                                  (ScalarE)
    eb [53, rows]  = broadcast of e via K=1 matmul
    logitsU.T [53, bags] = windowed reduce_16(S.T * eb)   (VectorE)
    z, 1/z, bias, final transpose to [bags, 53] on device.

x is transposed on the host during sharding (fp32 DMA-transpose is not
supported by the hardware xbar; a contiguous load of host-transposed x runs at
full HBM bandwidth).
"""

import os
from contextlib import ExitStack

import numpy as np

import concourse.bass as bass
import concourse.tile as tile
from concourse import bacc, mybir
from concourse.bass_utils import run_bass_kernel_spmd
from concourse.masks import make_identity

# Problem constants (hardcoded per spec nn_Attention_85478439125349)
N = 262144
B = 16384
D = 768
C = 53
BAG = 16
N_CORES = 8
ROWS = N // N_CORES          # 32768 sentences per core
BAGS = B // N_CORES          # 2048 bags per core
KCH = D // 128               # 6 contraction chunks
F32 = mybir.dt.float32


def build_nc(rows: int, sc: int = 2048, ch: int = 512) -> bass.Bass:
    """Build the per-core Bass program for `rows` sentences (bags of BAG)."""
    assert rows % sc == 0 and sc % ch == 0 and ch % BAG == 0
    bags = rows // BAG
    n_sc = rows // sc          # superchunks (DMA granularity)
    n_ch = sc // ch            # compute chunks per superchunk
    chb = ch // BAG            # bags per compute chunk (32)
    scb = sc // BAG            # bags per superchunk (128)

    nc = bacc.Bacc()
    xt = nc.declare_dram_parameter("xt", [D, rows], F32, isOutput=False)
    oht = nc.declare_dram_parameter("oht", [C, rows], F32, isOutput=False)
    relt = nc.declare_dram_parameter("relt", [D, C], F32, isOutput=False)
    biast = nc.declare_dram_parameter("biast", [C, 1], F32, isOutput=False)
    out = nc.declare_dram_parameter("out", [bags, C], F32, isOutput=True)

    xt_v = xt.rearrange("(k p) r -> k p r", p=128)        # [KCH, 128, rows]
    relt_v = relt.rearrange("(k p) c -> k p c", p=128)    # [KCH, 128, C]

    with tile.TileContext(nc) as tc, ExitStack() as ctx:
        consts = ctx.enter_context(tc.tile_pool(name="consts", bufs=1))
        xpool = ctx.enter_context(tc.tile_pool(name="xpool", bufs=2))
        ohpool = ctx.enter_context(tc.tile_pool(name="ohpool", bufs=2))
        work = ctx.enter_context(tc.tile_pool(name="work", bufs=2))
        psum = ctx.enter_context(tc.tile_pool(name="psum", bufs=2, space="PSUM"))
        psumc = ctx.enter_context(tc.tile_pool(name="psumc", bufs=1, space="PSUM"))

        # --- constants ---
        relt_sb = consts.tile([128, KCH, C], F32)
        nc.sync.dma_start(out=relt_sb, in_=relt_v.transpose([1, 0, 2]))
        bias_sb = consts.tile([C, 1], F32)
        nc.sync.dma_start(out=bias_sb, in_=biast[:, :])
        ones_c1 = consts.tile([C, 1], F32)
        nc.vector.memset(ones_c1, 1.0)
        ones_1c = consts.tile([1, C], F32)
        nc.vector.memset(ones_1c, 1.0)
        ident = consts.tile([128, 128], F32)
        make_identity(nc, ident)
        # accumulator for logits^T [C, bags] and staging for transposed output
        lt_acc = consts.tile([C, bags], F32)
        logits_sb = consts.tile([128, bags // 128, C], F32)

        # Never-read PSUM scratch, written only by PE. Warm-up matmuls write
        # here so the PE's vector clock passes every constant producer and
        # each steady-state matmul carries at most ONE sync wait (the walrus
        # LDWEIGHTS struct has a single wait slot).
        wu = psumc.tile([128, 128], F32)
        nc.tensor.matmul(wu[:1, :1], lhsT=ones_c1, rhs=ones_c1[:, 0:1])
        nc.tensor.matmul(
            wu[:C, :C], lhsT=relt_sb[:, 0, :], rhs=relt_sb[:, 0, :C]
        )
        nc.tensor.transpose(wu, ident, ident)

        for isc in range(n_sc):
            x_sb = xpool.tile([128, KCH, sc], F32)
            nc.sync.dma_start(
                out=x_sb,
                in_=xt_v[:, :, isc * sc : (isc + 1) * sc].transpose([1, 0, 2]),
            )
            oh_sb = ohpool.tile([C, sc], F32)
            nc.sync.dma_start(out=oh_sb, in_=oht[:, isc * sc : (isc + 1) * sc])
            # DMA-wait absorber: advances PE's DMA-sem clock so the first real
            # matmul of this superchunk needs only its PSUM-slot wait.
            nc.tensor.matmul(wu[:C, :1], lhsT=relt_sb[:, 0, :], rhs=x_sb[:, 0, 0:1])

            for ic in range(n_ch):
                cs = slice(ic * ch, (ic + 1) * ch)
                st = psum.tile([C, ch], F32, tag="st")
                for k in range(KCH):
                    nc.tensor.matmul(
                        st,
                        lhsT=relt_sb[:, k, :],
                        rhs=x_sb[:, k, cs],
                        start=(k == 0),
                        stop=(k == KCH - 1),
                    )
                # att extraction: sm = S^T * onehot^T ; att = ones53^T @ sm
                sm = work.tile([C, ch], F32, tag="sm")
                nc.vector.tensor_mul(sm, st, oh_sb[:, cs])
                att = psum.tile([1, ch], F32, tag="att")
                nc.tensor.matmul(att, lhsT=ones_c1, rhs=sm)
                # copy att to SBUF on DVE so the att PSUM slot's release sem is
                # DVE (collapses with the matmul's rhs wait next chunk)
                att_sb = work.tile([1, ch], F32, tag="att_sb")
                nc.vector.tensor_copy(att_sb, att)
                e = work.tile([1, ch], F32, tag="e")
                nc.scalar.activation(e, att_sb, mybir.ActivationFunctionType.Exp)
                # broadcast e across 53 partitions, weight S^T, bag-reduce
                eb = psum.tile([C, ch], F32, tag="eb")
                nc.tensor.matmul(eb, lhsT=ones_1c, rhs=e)
                ebs = work.tile([C, ch], F32, tag="ebs")
                nc.scalar.copy(ebs, eb)
                w = work.tile([C, ch], F32, tag="w")
                nc.vector.tensor_mul(w, st, ebs)
                lu = work.tile([C, chb], F32, tag="lu")
                nc.vector.reduce_sum(
                    lu, w.rearrange("p (b j) -> p b j", j=BAG), axis=mybir.AxisListType.X
                )
                # z per bag from the broadcast copy; normalize
                zb = work.tile([C, chb], F32, tag="zb")
                nc.vector.reduce_sum(
                    zb,
                    ebs.rearrange("p (b j) -> p b j", j=BAG),
                    axis=mybir.AxisListType.X,
                )
                rzb = work.tile([C, chb], F32, tag="rzb")
                nc.vector.reciprocal(rzb, zb)
                ob = isc * scb + ic * chb
                nc.vector.tensor_mul(lt_acc[:, ob : ob + chb], lu, rzb)
            # bias for this superchunk's bag block
            bs = slice(isc * scb, (isc + 1) * scb)
            nc.vector.tensor_scalar_add(
                out=lt_acc[:, bs], in0=lt_acc[:, bs], scalar1=bias_sb
            )

        # transpose logits^T [C, bags] -> [bags, C] and store
        for t in range(bags // 128):
            pt = psum.tile([128, C], F32, tag="pt", bufs=1)
            nc.tensor.transpose(
                pt, lt_acc[:, t * 128 : (t + 1) * 128], ident[:C, :C]
            )
            nc.vector.tensor_copy(logits_sb[:, t, :], pt)
        nc.sync.dma_start(
            out=out.rearrange("(t p) c -> p t c", p=128), in_=logits_sb
        )
    return nc


_NC_CACHE: dict = {}


def _get_nc(rows: int) -> bass.Bass:
    if rows not in _NC_CACHE:
        nc = build_nc(rows)
        nc.finalize()
        _NC_CACHE[rows] = nc
    return _NC_CACHE[rows]


def _numpy_fallback(x, rel_weight, bias, input_scope, query):
    """Pure-numpy replication of the reference for non-uniform bag layouts."""
    n = x.shape[0]
    num_bags = input_scope.shape[0] - 1
    seg = np.searchsorted(input_scope[1:], np.arange(n), side="right")
    att = np.einsum("nd,nd->n", x, rel_weight[query]).astype(np.float32)
    valid = seg < num_bags
    segv = seg[valid]
    attv = att[valid]
    m = np.full(num_bags, -np.inf, dtype=np.float32)
    np.maximum.at(m, segv, attv)
    e = np.zeros(n, dtype=np.float32)
    e[valid] = np.exp(attv - m[segv])
    z = np.zeros(num_bags, dtype=np.float32)
    np.add.at(z, segv, e[valid])
    w = np.zeros(n, dtype=np.float32)
    nz = z[segv] != 0
    w_valid = np.zeros(segv.shape[0], dtype=np.float32)
    w_valid[nz] = e[valid][nz] / z[segv][nz]
    w[valid] = w_valid
    repre = np.zeros((num_bags, x.shape[1]), dtype=np.float32)
    np.add.at(repre, segv, (x[valid] * w[valid][:, None]).astype(np.float32))
    return repre @ rel_weight.T + bias


def _prepare_in_maps(x, rel_weight, bias, query):
    relt = np.ascontiguousarray(rel_weight.T).astype(np.float32)  # [D, C]
    biast = np.ascontiguousarray(bias.reshape(C, 1)).astype(np.float32)
    q = query.astype(np.int64)
    in_maps = []
    for c in range(N_CORES):
        lo, hi = c * ROWS, (c + 1) * ROWS
        xt = np.ascontiguousarray(x[lo:hi].T)  # [D, ROWS]
        oh = np.zeros((C, ROWS), dtype=np.float32)
        oh[q[lo:hi], np.arange(ROWS)] = 1.0
        in_maps.append({"xt": xt, "oht": oh, "relt": relt, "biast": biast})
    return in_maps


def run_device(x, rel_weight, bias, query, trace=False, **kwargs):
    nc = _get_nc(ROWS)
    in_maps = _prepare_in_maps(x, rel_weight, bias, query)
    res = run_bass_kernel_spmd(
        nc, in_maps, core_ids=list(range(N_CORES)), trace=trace, **kwargs
    )
    outs = [np.asarray(r["out"]) for r in res.results]
    return np.concatenate(outs, axis=0), res


def kernel(x, rel_weight, bias, input_scope, query):
    x = np.asarray(x, dtype=np.float32)
    rel_weight = np.asarray(rel_weight, dtype=np.float32)
    bias = np.asarray(bias, dtype=np.float32)
    input_scope = np.asarray(input_scope)
    query = np.asarray(query)

    expected_scope = np.arange(B + 1, dtype=np.int64) * (N // B)
    if (
        x.shape == (N, D)
        and rel_weight.shape == (C, D)
        and input_scope.shape == (B + 1,)
        and np.array_equal(input_scope.astype(np.int64), expected_scope)
    ):
        out, _ = run_device(x, rel_weight, bias, query)
        return out
    return _numpy_fallback(x, rel_weight, bias, input_scope, query)
